# revision 2
# baseline (speedup 1.0000x reference)
"""Trainium2 Bass kernel v3 for nn_GatGraphClassifier.

Sharding (same as baseline): 8 cores = 4 graphs x 2 head-groups; layer 1
computed redundantly per pair; layer 2 split by heads; host combines.

Key optimizations vs baseline:
  - Rank-1 score factorization: exp(leaky(s_i+t_j)) = v_j * A'_ij with
    A'_ij = max(q_i, w_j) * u_i  where q=exp(.8 s), u=exp(.2 s),
    v=exp(t), w=exp(-.8 t).  One DVE/Pool STT pass builds each A' tile
    (vs 2 ACT exps + DVE max over NxN in baseline).  v folds into the
    projection eviction (ACT Copy w/ per-partition scale), and the ones
    column of p' becomes v so the attention matmul's 257th column still
    yields the softmax row-sum for free.
  - fp8e4 (e4m3) DoubleRow attention matmuls: A' and p' stored fp8,
    contraction 256/matmul at 0.5 cyc/row.
  - elu(z) = min(z, exp(z)-1): 1 ACT + 1 DVE pass (vs 2+2).
  - Engine-balanced evictions (ACT does psum->sbuf copies w/ scale,
    DVE/Pool split the A'-builds), pipelined emission: skip-proj and ELU
    interleaved into the attention head loop to keep PE busy.
"""

import numpy as np
import ml_dtypes

import concourse.bass as bass
import concourse.tile as tile
from concourse import bacc, mybir
from concourse.bass_utils import run_bass_kernel_spmd

dt = mybir.dt
AF = mybir.ActivationFunctionType
ALU = mybir.AluOpType
PM = mybir.MatmulPerfMode
BF16 = ml_dtypes.bfloat16
FP8 = ml_dtypes.float8_e4m3

B, N, H, F, DIN, NCLS = 4, 1024, 8, 256, 256, 10
# p' = lam * exp(t) * p and A~ = mu * max(q, w) are stored fp8e4
# (IEEE e4m3: max 240, has inf).  lam and mu keep them in range; both cancel
# exactly in the softmax normalization (they scale numerator and denominator).
LOG_LAM1 = float(np.log(1.0 / 4.0))
LOG_LAM2 = float(np.log(1.0 / 32.0))
MU = {1: 0.5, 2: 0.125}
NCORES = 8
G = H // 2
C1 = H * F
P = 128

CFG = dict(n=N, h1=H, g2=G, f=F, din=DIN)


DEBUG = False


def build_program(cfg=None, has_mask=False, has_b1=False):
    cfg = dict(CFG if cfg is None else cfg)
    n, h1, g2, f, din = cfg["n"], cfg["h1"], cfg["g2"], cfg["f"], cfg["din"]
    c1 = h1 * f
    nb_ = n // P
    w2cols = g2 * f + f + 2 * g2

    nc = bacc.Bacc("TRN2", target_bir_lowering=False, debug=False,
                   num_devices=NCORES)
    d = {}
    xrows = din + (1 if has_b1 else 0)
    d["xT"] = nc.dram_tensor("xT", [din, n], dt.bfloat16, kind="ExternalInput").ap()
    if has_b1:
        d["xTones"] = nc.dram_tensor("xTones", [1, n], dt.bfloat16, kind="ExternalInput").ap()
    d["w1p"] = nc.dram_tensor("w1p", [din, c1 + 2 * h1], dt.bfloat16, kind="ExternalInput").ap()
    d["w1s"] = nc.dram_tensor("w1s", [din + (1 if has_b1 else 0), c1], dt.bfloat16, kind="ExternalInput").ap()
    d["w2e"] = nc.dram_tensor("w2e", [c1, w2cols], dt.bfloat16, kind="ExternalInput").ap()
    d["idf"] = nc.dram_tensor("idf", [P, P], dt.float32, kind="ExternalInput").ap()
    d["idb"] = nc.dram_tensor("idb", [P, P], dt.bfloat16, kind="ExternalInput").ap()
    if has_mask:
        d["expmT"] = nc.dram_tensor("expmT", [n, n], dt.bfloat16, kind="ExternalInput").ap()
    d["gpart"] = nc.dram_tensor("gpart", [1, f], dt.float32, kind="ExternalOutput").ap()
    if DEBUG:
        d["d_vw1"] = nc.dram_tensor("d_vw1", [P, nb_ * 2 * h1], dt.float32, kind="ExternalOutput").ap()
        d["d_quT1"] = nc.dram_tensor("d_quT1", [h1, n], dt.float32, kind="ExternalOutput").ap()
        d["d_p1"] = nc.dram_tensor("d_p1", [P, h1 * nb_ * (f + 1)], dt.float8e4, kind="ExternalOutput").ap()
        d["d_A"] = nc.dram_tensor("d_A", [P, nb_ * n], dt.float8e4, kind="ExternalOutput").ap()
        d["d_x1"] = nc.dram_tensor("d_x1", [P, c1], dt.bfloat16, kind="ExternalOutput").ap()
        d["d_x1fT"] = nc.dram_tensor("d_x1fT", [P, n], dt.bfloat16, kind="ExternalOutput").ap()
        d["d_out2"] = nc.dram_tensor("d_out2", [P, nb_ * f], dt.float32, kind="ExternalOutput").ap()

    with tile.TileContext(nc) as tc:
        _emit(tc, cfg, has_mask, has_b1, d)
    nc.compile()
    return nc


def _emit(tc, cfg, has_mask, has_b1, d):
    nc = tc.nc
    n, h1, g2, f, din = cfg["n"], cfg["h1"], cfg["g2"], cfg["f"], cfg["din"]
    c1 = h1 * f
    nb = n // P
    kt1 = din // P
    kt2 = c1 // P
    ct = c1 // P
    fp1 = f + 1
    w2cols = g2 * f + f + 2 * g2
    em_d = d.get("expmT")

    with tc.tile_pool(name="pp", bufs=1) as pp:
        # ---------------- persistent tiles ----------------
        xT = []
        for k in range(kt1):
            t = pp.tile([P, n], dt.bfloat16, tag=f"xT{k}", name=f"xT{k}")
            nc.sync.dma_start(t[:], d["xT"][k * P:(k + 1) * P, :])
            xT.append(t)
        if has_b1:
            xTones = pp.tile([1, n], dt.bfloat16, tag="xTones", name="xTones")
            nc.sync.dma_start(xTones[:], d["xTones"][:])
        idf = pp.tile([P, P], dt.float32, tag="idf", name="idf")
        nc.sync.dma_start(idf[:], d["idf"][:])
        idb = pp.tile([P, P], dt.bfloat16, tag="idb", name="idb")
        nc.sync.dma_start(idb[:], d["idb"][:])

        quT1 = pp.tile([h1, n], dt.float32, tag="quT1", name="quT1")
        vw1 = pp.tile([P, nb * 2 * h1], dt.float32, tag="vw1", name="vw1")
        vw1v = vw1[:].rearrange("p (k c) -> p k c", c=2 * h1)
        # p' per head: [p, h, k(jt), fp1] fp8
        p1 = pp.tile([P, h1 * nb * fp1], dt.float8e4, tag="p1", name="p1")
        p1v = p1[:].rearrange("p (h k c) -> p h k c", k=nb, c=fp1)

        x1pre = [pp.tile([P, c1], dt.bfloat16, tag=f"x1p{i}", name=f"x1p{i}")
                 for i in range(nb)]
        out2 = [pp.tile([P, f], dt.float32, tag=f"o2_{i}", name=f"o2_{i}")
                for i in range(nb)]
        onesc = pp.tile([P, 1], dt.float32, tag="ones", name="ones")
        nc.vector.memset(onesc[:], 1.0)
        lamb1 = pp.tile([P, 1], dt.float32, tag="lamb1", name="lamb1")
        nc.vector.memset(lamb1[:], LOG_LAM1)
        lamb2 = pp.tile([P, 1], dt.float32, tag="lamb2", name="lamb2")
        nc.vector.memset(lamb2[:], LOG_LAM2)

        quT2 = pp.tile([g2, n], dt.float32, tag="quT2", name="quT2")
        vw2 = pp.tile([P, nb * 2 * g2], dt.float32, tag="vw2", name="vw2")
        vw2v = vw2[:].rearrange("p (k c) -> p k c", c=2 * g2)
        p2 = pp.tile([P, g2 * nb * fp1], dt.float8e4, tag="p2", name="p2")
        p2v = p2[:].rearrange("p (h k c) -> p h k c", k=nb, c=fp1)

        em_tiles = None
        if has_mask:
            em_tiles = []
            for jt in range(nb):
                em = pp.tile([P, n], dt.bfloat16, tag=f"em{jt}", name=f"em{jt}")
                nc.sync.dma_start(em[:], em_d[jt * P:(jt + 1) * P, :])
                em_tiles.append(em)

        # ================= L1 projection =================
        with tc.tile_pool(name="pW1", bufs=1) as pW1:
            w1pt = []
            for k in range(kt1):
                t = pW1.tile([P, c1 + 2 * h1], dt.bfloat16, tag=f"w1p{k}", name=f"w1p{k}")
                nc.sync.dma_start(t[:], d["w1p"][k * P:(k + 1) * P, :])
                w1pt.append(t)
            w1st = []
            for k in range(kt1):
                t = pW1.tile([P, c1], dt.bfloat16, tag=f"w1s{k}", name=f"w1s{k}")
                nc.sync.dma_start(t[:], d["w1s"][k * P:(k + 1) * P, :])
                w1st.append(t)
            if has_b1:
                w1sb = pW1.tile([1, c1], dt.bfloat16, tag="w1sb", name="w1sb")
                nc.sync.dma_start(w1sb[:], d["w1s"][din:din + 1, :])

            with tc.tile_pool(name="pqv", bufs=2) as pqv, \
                 tc.tile_pool(name="psS", bufs=1, space="PSUM") as psS, \
                 tc.tile_pool(name="psP", bufs=2, space="PSUM") as psP:
                for ib in range(nb):
                    isl = slice(ib * P, (ib + 1) * P)
                    # s projection -> exps
                    pss = psS.tile([P, 2 * h1], dt.float32, tag="sp", name="sp")
                    for k in range(kt1):
                        nc.tensor.matmul(pss[:], xT[k][:, isl],
                                         w1pt[k][:, c1:c1 + 2 * h1],
                                         start=(k == 0), stop=(k == kt1 - 1))
                    quv = pqv.tile([P, h1], dt.float32, tag="quv", name="quv")
                    nc.scalar.activation(vw1v[:, ib, 0:h1], pss[:, h1:2 * h1],
                                         AF.Exp, bias=lamb1[:])
                    nc.scalar.activation(vw1v[:, ib, h1:2 * h1], pss[:, h1:2 * h1],
                                         AF.Exp, scale=-0.8)
                    nc.scalar.activation(quv[:], pss[:, 0:h1], AF.Exp,
                                         scale=0.8)
                    psq = psS.tile([h1, P], dt.float32, tag="quvT", name="quvT")
                    nc.tensor.transpose(psq[:], quv[:], idf[:])
                    nc.vector.tensor_copy(quT1[:, isl], psq[:])
                    # main projection, scaled eviction (p' = v * p)
                    for hp in range(0, h1, 2):
                        po = psP.tile([P, 2 * f], dt.float32, tag="proj", name="proj")
                        for k in range(kt1):
                            nc.tensor.matmul(po[:], xT[k][:, isl],
                                             w1pt[k][:, hp * f:(hp + 2) * f],
                                             start=(k == 0), stop=(k == kt1 - 1))
                        nc.scalar.activation(p1v[:, hp, ib, 0:f], po[:, 0:f],
                                             AF.Copy, scale=vw1v[:, ib, hp:hp + 1])
                        nc.scalar.activation(p1v[:, hp + 1, ib, 0:f], po[:, f:2 * f],
                                             AF.Copy,
                                             scale=vw1v[:, ib, hp + 1:hp + 2])
                    # ones-cols: p'[:, h, ib, 256] = v_h (strided copy over h)
                    nc.vector.tensor_copy(p1v[:, :, ib, f:fp1], vw1v[:, ib, 0:h1])

                if DEBUG:
                    nc.sync.dma_start(d["d_vw1"][:], vw1[:])
                    nc.sync.dma_start(d["d_quT1"][:], quT1[:])
                    nc.sync.dma_start(d["d_p1"][:], p1[:])

            # ============ L1 attention + skip + ELU (interleaved) ======
            with tc.tile_pool(name="pSE", bufs=2) as pSE, \
                 tc.tile_pool(name="psPb", bufs=2, space="PSUM") as psPb:
                _attention_phase(
                    tc, nc, heads=h1, nb=nb, n=n, f=f, fp1=fp1,
                    quT=quT1, vwv=vw1v, pv=p1v, dst=x1pre, dst_accum=False,
                    em_tiles=em_tiles, layer=1,
                    skip_fn=lambda h: _skip_chunk(
                        tc, nc, h, nb, c1, kt1, xT, w1st,
                        w1sb if has_b1 else None,
                        xTones if has_b1 else None, x1pre, psPb, pSE),
                    dbg=d if DEBUG else None,
                )
                if DEBUG:
                    nc.sync.dma_start(d["d_x1"][:], x1pre[0][:])

        # ================= transpose x1 -> x1fT =================
        with tc.tile_pool(name="pE", bufs=1) as pE:
            x1fT = [pE.tile([P, n], dt.bfloat16, tag=f"xT2_{c}", name=f"xT2_{c}")
                    for c in range(ct)]
            with tc.tile_pool(name="psE", bufs=2, space="PSUM") as psE:
                for cb in range(ct):
                    for i0 in range(0, nb, 4):
                        po = psE.tile([P, 4 * P], dt.bfloat16, tag="tr", name="tr")
                        for q in range(4):
                            nc.tensor.transpose(
                                po[:, q * P:(q + 1) * P],
                                x1pre[i0 + q][:, cb * P:(cb + 1) * P], idb[:])
                        nc.scalar.copy(x1fT[cb][:, i0 * P:(i0 + 4) * P], po[:])
            if DEBUG:
                nc.sync.dma_start(d["d_x1fT"][:], x1fT[0][:])

            # ================= L2 =================
            with tc.tile_pool(name="pW2", bufs=1) as pW2:
                w2et = []
                for k in range(kt2):
                    t = pW2.tile([P, w2cols], dt.bfloat16, tag=f"w2e{k}", name=f"w2e{k}")
                    nc.sync.dma_start(t[:], d["w2e"][k * P:(k + 1) * P, :])
                    w2et.append(t)

                with tc.tile_pool(name="pqv2", bufs=2) as pqv2, \
                     tc.tile_pool(name="psS2", bufs=1, space="PSUM") as psS2, \
                     tc.tile_pool(name="psP2", bufs=2, space="PSUM") as psP2:
                    gf = g2 * f
                    for ib in range(nb):
                        isl = slice(ib * P, (ib + 1) * P)
                        pss = psS2.tile([P, f + 2 * g2], dt.float32, tag="sp2", name="sp2")
                        for k in range(kt2):
                            nc.tensor.matmul(pss[:], x1fT[k][:, isl],
                                             w2et[k][:, gf:gf + f + 2 * g2],
                                             start=(k == 0), stop=(k == kt2 - 1))
                        quv = pqv2.tile([P, g2], dt.float32, tag="quv2", name="quv2")
                        nc.scalar.activation(vw2v[:, ib, 0:g2],
                                             pss[:, f + g2:f + 2 * g2], AF.Exp,
                                             bias=lamb2[:])
                        nc.scalar.activation(vw2v[:, ib, g2:2 * g2],
                                             pss[:, f + g2:f + 2 * g2], AF.Exp,
                                             scale=-0.8)
                        nc.scalar.activation(quv[:], pss[:, f:f + g2],
                                             AF.Exp, scale=0.8)
                        psq = psS2.tile([g2, P], dt.float32, tag="quvT2", name="quvT2")
                        nc.tensor.transpose(psq[:], quv[:], idf[:])
                        nc.vector.tensor_copy(quT2[:, isl], psq[:])
                        # skip2sum -> out2 init
                        nc.vector.tensor_copy(out2[ib][:], pss[:, 0:f])
                        for hp in range(0, g2, 2):
                            po = psP2.tile([P, 2 * f], dt.float32, tag="proj2", name="proj2")
                            for k in range(kt2):
                                nc.tensor.matmul(po[:], x1fT[k][:, isl],
                                                 w2et[k][:, hp * f:(hp + 2) * f],
                                                 start=(k == 0), stop=(k == kt2 - 1))
                            nc.scalar.activation(p2v[:, hp, ib, 0:f], po[:, 0:f],
                                                 AF.Copy,
                                                 scale=vw2v[:, ib, hp:hp + 1])
                            nc.scalar.activation(p2v[:, hp + 1, ib, 0:f],
                                                 po[:, f:2 * f], AF.Copy,
                                                 scale=vw2v[:, ib, hp + 1:hp + 2])
                        nc.vector.tensor_copy(p2v[:, :, ib, f:fp1], vw2v[:, ib, 0:g2])

                    _attention_phase(
                        tc, nc, heads=g2, nb=nb, n=n, f=f, fp1=fp1,
                        quT=quT2, vwv=vw2v, pv=p2v, dst=out2, dst_accum=True,
                        em_tiles=em_tiles, layer=2, skip_fn=None)
                    if DEBUG:
                        for ib in range(nb):
                            nc.sync.dma_start(d["d_out2"][:, ib * f:(ib + 1) * f],
                                              out2[ib][:])

        # ================= pooling =================
        with tc.tile_pool(name="psH", bufs=1, space="PSUM") as psH, \
             tc.tile_pool(name="pH", bufs=1) as pH:
            pg = psH.tile([1, f], dt.float32, tag="pool", name="pool")
            for ib in range(nb):
                nc.tensor.matmul(pg[:], onesc[:], out2[ib][:],
                                 start=(ib == 0), stop=(ib == nb - 1))
            gout = pH.tile([1, f], dt.float32, tag="g", name="g")
            nc.vector.tensor_copy(gout[:], pg[:])
            nc.sync.dma_start(d["gpart"][:], gout[:])


def _skip_chunk(tc, nc, h, nb, c1, kt1, xT, w1st, w1sb, xTones, x1pre, psP, pSE):
    """After heads h-1,h finish: skip-proj + add + ELU for cols of those heads."""
    if h % 2 == 0:
        return
    f = 256
    cc = (h // 2) * 2 * f
    w = 2 * f
    for ib in range(nb):
        isl = slice(ib * P, (ib + 1) * P)
        po = psP.tile([P, w], dt.float32, tag="proj", name="skp")
        for k in range(kt1):
            nc.tensor.matmul(po[:], xT[k][:, isl], w1st[k][:, cc:cc + w],
                             start=(k == 0), stop=(w1sb is None and k == kt1 - 1))
        if w1sb is not None:
            nc.tensor.matmul(po[:], xTones[:, isl], w1sb[:, cc:cc + w],
                             start=False, stop=True)
        sl = x1pre[ib][:, cc:cc + w]
        nc.vector.scalar_tensor_tensor(out=sl, in0=po[:], scalar=0.0, in1=sl,
                                       op0=ALU.add, op1=ALU.add)
        # elu(z) = min(relu(z), exp(z)-1)
        e = pSE.tile([P, w], dt.bfloat16, tag="elu", name="elu")
        nc.scalar.activation(e[:], sl, AF.Exp)
        r = pSE.tile([P, w], dt.bfloat16, tag="elur", name="elur")
        nc.scalar.activation(r[:], sl, AF.Relu)
        nc.vector.scalar_tensor_tensor(out=sl, in0=e[:], scalar=-1.0, in1=r[:],
                                       op0=ALU.add, op1=ALU.min)


def _attention_phase(tc, nc, *, heads, nb, n, f, fp1, quT, vwv, pv, dst,
                     dst_accum, em_tiles, layer, skip_fn, dbg=None):
    """Per head: broadcast q row, build A~ = max(q_i, w_j) tiles (the
    exp(.2 s_i) factor cancels in the softmax), fp8-DR matmuls, evict."""
    with tc.tile_pool(name=f"pA{layer}", bufs=2) as pA, \
         tc.tile_pool(name=f"pB{layer}", bufs=2) as pB, \
         tc.tile_pool(name=f"pR{layer}", bufs=4) as pR, \
         tc.tile_pool(name=f"psB{layer}", bufs=4, space="PSUM") as psB:
        for h in range(heads):
            qrow = pB.tile([1, n], dt.float32, tag="qrow", name="qrow")
            nc.sync.dma_start(qrow[:], quT[h:h + 1, :])
            qb = pB.tile([P, n], dt.float32, tag="qb", name="qb")
            nc.gpsimd.partition_broadcast(qb[:], qrow[:])
            A = pA.tile([P, nb * n], dt.float8e4, tag="A", name=f"A{layer}_{h}")
            Av = A[:].rearrange("p (k i) -> p k i", i=n)
            mu = MU[layer]
            for jt in range(nb):
                if em_tiles is None:
                    nc.vector.tensor_scalar(
                        out=Av[:, jt, :], in0=qb[:],
                        scalar1=vwv[:, jt, heads + h:heads + h + 1],
                        scalar2=mu, op0=ALU.max, op1=ALU.mult)
                else:
                    tmp = pR.tile([P, n], dt.bfloat16, tag="am", name="am")
                    nc.vector.tensor_scalar(
                        out=tmp[:], in0=qb[:],
                        scalar1=vwv[:, jt, heads + h:heads + h + 1],
                        scalar2=mu, op0=ALU.max, op1=ALU.mult)
                    nc.vector.tensor_tensor(Av[:, jt, :], tmp[:],
                                            em_tiles[jt][:], op=ALU.mult)
            if dbg is not None and layer == 1 and h == 0:
                nc.sync.dma_start(dbg["d_A"][:], A[:])
            for ibg in range(0, nb, 4):
                pos = [psB.tile([P, fp1], dt.float32, tag="attn", name="attn")
                       for _ in range(4)]
                for kp in range(nb // 2):
                    for q in range(4):
                        ib = ibg + q
                        nc.tensor.matmul(
                            pos[q][:],
                            Av[:, 2 * kp:2 * kp + 2, ib * P:(ib + 1) * P],
                            pv[:, h, 2 * kp:2 * kp + 2, :],
                            start=(kp == 0), stop=(kp == nb // 2 - 1),
                            perf_mode=PM.DoubleRow)
                for q in range(4):
                    ib = ibg + q
                    po = pos[q]
                    rc = pR.tile([P, 1], dt.float32, tag="rc", name="rc")
                    nc.vector.reciprocal(rc[:], po[:, f:f + 1])
                    if dst_accum:
                        nc.vector.scalar_tensor_tensor(
                            out=dst[ib][:], in0=po[:, 0:f], scalar=rc[:],
                            in1=dst[ib][:], op0=ALU.mult, op1=ALU.add)
                    else:
                        nc.scalar.activation(dst[ib][:, h * f:(h + 1) * f],
                                             po[:, 0:f], AF.Copy, scale=rc[:])
            if skip_fn is not None:
                skip_fn(h)


# ---------------------------------------------------------------------------
# host side
# ---------------------------------------------------------------------------

_COMPILED = {}


def _get_program(has_mask, has_b1):
    key = (bool(has_mask), bool(has_b1))
    if key not in _COMPILED:
        _COMPILED[key] = build_program(has_mask=key[0], has_b1=key[1])
    return _COMPILED[key]


def make_host_inputs(features, attn_mask, W1, a_src1, a_tgt1, skip1, b1,
                     W2, a_src2, a_tgt2, skip2, has_mask, has_b1, cfg=None):
    cfg = dict(CFG if cfg is None else cfg)
    n, h1, g2, f, din = cfg["n"], cfg["h1"], cfg["g2"], cfg["f"], cfg["din"]
    c1 = h1 * f
    ngrp = h1 // g2
    f32 = np.float32
    W1 = np.asarray(W1, f32); skip1 = np.asarray(skip1, f32)
    W2 = np.asarray(W2, f32); skip2 = np.asarray(skip2, f32)
    b1 = np.asarray(b1, f32)
    wsrc1 = np.einsum("dhf,hf->dh", W1.reshape(din, h1, f), np.asarray(a_src1, f32))
    wtgt1 = np.einsum("dhf,hf->dh", W1.reshape(din, h1, f), np.asarray(a_tgt1, f32))
    w1p = np.concatenate([W1, wsrc1, wtgt1], axis=1).astype(BF16)
    if has_b1:
        w1s = np.concatenate([skip1, b1.reshape(1, c1)], axis=0).astype(BF16)
    else:
        w1s = skip1.astype(BF16)
    idf = np.eye(P, dtype=f32)
    idb = np.eye(P).astype(BF16)

    w2e_g = []
    for g in range(ngrp):
        cols = slice(g * g2 * f, (g + 1) * g2 * f)
        w2g = W2[:, cols]
        a_s = np.asarray(a_src2, f32)[g * g2:(g + 1) * g2]
        a_t = np.asarray(a_tgt2, f32)[g * g2:(g + 1) * g2]
        wsrc2 = np.einsum("dkf,kf->dk", w2g.reshape(c1, g2, f), a_s)
        wtgt2 = np.einsum("dkf,kf->dk", w2g.reshape(c1, g2, f), a_t)
        sk2sum = skip2[:, cols].reshape(c1, g2, f).sum(axis=1)
        w2e_g.append(np.concatenate([w2g, sk2sum, wsrc2, wtgt2], axis=1).astype(BF16))

    nbatch = np.asarray(features).shape[0]
    in_maps = []
    for c in range(nbatch * ngrp):
        b = c // ngrp
        g = c % ngrp
        xT = np.ascontiguousarray(np.asarray(features[b], f32).T)
        m = dict(xT=xT.astype(BF16), w1p=w1p, w1s=w1s, w2e=w2e_g[g],
                 idf=idf, idb=idb)
        if has_b1:
            m["xTones"] = np.ones((1, n), BF16)
        if has_mask:
            mT = np.ascontiguousarray(np.asarray(attn_mask[b], f32).T)
            m["expmT"] = np.exp(np.maximum(mT, -80.0)).astype(BF16)
        in_maps.append(m)
    return in_maps


def finish_host(results, b2, Wc, bc, cfg=None):
    cfg = dict(CFG if cfg is None else cfg)
    n, h1, g2 = cfg["n"], cfg["h1"], cfg["g2"]
    ngrp = h1 // g2
    b2 = np.asarray(b2, np.float64)
    Wc = np.asarray(Wc, np.float64)
    bc = np.asarray(bc, np.float64)
    nbatch = len(results) // ngrp
    out = np.zeros((nbatch, Wc.shape[1]), np.float64)
    for b in range(nbatch):
        gsum = sum(results[b * ngrp + g]["gpart"][0].astype(np.float64)
                   for g in range(ngrp))
        gv = gsum / (h1 * n) + b2
        out[b] = gv @ Wc + bc
    return out.astype(np.float32)


def kernel(features, eigvects, attn_mask, W1, a_src1, a_tgt1, skip1, b1,
           W2, a_src2, a_tgt2, skip2, b2, Wc, bc):
    has_mask = bool(np.any(np.asarray(attn_mask)))
    has_b1 = bool(np.any(np.asarray(b1)))
    nc = _get_program(has_mask, has_b1)
    in_maps = make_host_inputs(features, attn_mask, W1, a_src1, a_tgt1, skip1,
                               b1, W2, a_src2, a_tgt2, skip2, has_mask, has_b1)
    res = run_bass_kernel_spmd(nc, in_maps, list(range(NCORES)))
    return finish_host(res.results, b2, Wc, bc)
